# revision 67
# baseline (speedup 1.0000x reference)
"""Fused attention kernel for Trainium2 (Bass/Tile), 8-core data-parallel.

Problem (nn_AttentionModel): B=8, L=2048, V=1024, D=512
    q = x @ Wq.T ; k = x @ Wk.T ; v = x @ Wv.T          (per batch element)
    out = softmax(q @ k.T / sqrt(D)) @ v
Sharding: data-parallel over batch - core b gets x[b] plus replicated
weights, computes its full attention on-chip, no collectives.

This revision (194us -> ~189us) adds startup + tail latency work on top
of the v1 design (host-side bf16/layout prep, flash-style streaming,
fp8 double-pumped scores - see kernel_baseline.py for that analysis).
Measured context: exec time runs from the "main" scope start to the
last instruction event; a fixed ~8.7us endgame (semaphore-clear storm,
DMA sweep, profiler flush) follows the last store regardless of kernel
structure, so the levers are (a) when useful TensorE work starts,
(b) keeping the 769-MM stream gapless, (c) when the last store lands.

1. Packed input DMA: all inputs ship as ONE DRAM tensor xin [P, 28, V]
   ordered [x_chunk0 | Wk | Wq | Wv | x_chunks1-3] in 5 sliced
   dma_starts in consumption order. Chunk-0 projections can start when
   slice [0:5] (x0 + Wk, 2.5MB... first 1.25MB critical) lands ~7.2us
   into exec. DMA descriptors run ~700ns cold vs ~270ns warm (engine
   clock ramp), so the startup transfer is ~3us/MB; finer slicing or a
   warm-up prefetch DMA measured WORSE (extra serial DIRECT2D gens).
2. Warm burst = 12 MMs sized to end exactly at chunk-0-DMA-complete
   (TensorE MMs ramp 600->216ns over ~3.5us; gaps at the start drop
   the chip-wide boost clock).
3. Fused final pair+finalize (the kernel tail): the last (qm=3, g=3)
   pair interleaves score and AV matmuls j-major (software-pipelined
   by one j), qs=2,3 PSUM accumulators are pre-initialized with the
   prior avacc partials so their tail is scale-from-PSUM, the bf16
   denominator cast runs on ScalarE right after the last exp, the four
   FD=1 ones-matmuls follow the last AV matmul, and the q-tiles scale
   across DVE (tensor_scalar muls) and ScalarE (activation scales)
   into two batched ships (four per-qs D2Ds left the last one
   sync-queue-gated, ~600ns serial each). Last store ~last-MM+4.2us.
   NOTE the tile scheduler is list-scheduling by earliest-ready time,
   NOT emission order - ops must be steered via engine choice and
   data readiness, and interleaved multi-column PSUM accumulation
   chains with manual start/stop get mangled (only canonical groups
   are safe; cost one failed revision each to learn).
4. Output stores in bf16 to a [P, NQM, NQT, D] DRAM layout (each
   partition writes contiguous bytes; host transposes + upcasts).
   Costs 0.03e-2 rel-err (simulated + verified), halves store bytes,
   and doubles DVE scale throughput. Earlier q-block finalizes batch
   all 4 tiles into one DMA; they are off the critical path.
5. Engine rebalance of the softmax denominator: regular pairs
   accumulate it entirely on GpSimd (full-width adds, ~87% gp busy),
   leaving DVE's queue holding only the avacc PSUM drains - with the
   old gp/DVE half-split, DVE lagged the 3-deep avp PSUM ring by
   26-517ns and the next pair's first AV matmul stalled ~220ns at
   every chunk transition (WAR on the psum bank). The three pairs
   whose finalize follows immediately keep the low-latency split
   (the finalize cast + FD1s would otherwise stall TensorE ~0.8us on
   the ~4.7us serial gp chain). Sharing the mm PSUM ring with pair
   accumulators instead measured catastrophically (+39us).

PSUM rings: mm 2 (projections) + sc 3 (scores) + av 3 (AV/Z) = 8 banks.
In the final pair the three avp banks hold pa[qs=0..2] simultaneously
and pa[qs=3] + zps borrow the mm ring (projections are done by then).
"""

import math
import sys

sys.path.insert(0, "/opt/trn_rl_repo")

import numpy as np
import ml_dtypes

import concourse.bacc as bacc
import concourse.bass as bass
import concourse.tile as tile
from concourse import mybir
from concourse.bass_utils import run_bass_kernel_spmd

B, L, V, D = 8, 2048, 1024, 512
P = 128
LT, VT, DT = L // P, V // P, D // P      # 16, 8, 4
QM = 512                                  # q columns per q-block
NQM = L // QM                             # 4 q-blocks == 4 chunks
NQT = QM // P                             # 4 q-tiles per block
CHT = 4                                   # l-tiles per chunk
SCALE = 1.0 / math.sqrt(D)
WARM = 12                                 # warm-burst MMs (cover chunk-0 DMA)

F32 = mybir.dt.float32
BF16 = mybir.dt.bfloat16
FP8 = mybir.dt.float8e4
DR = mybir.MatmulPerfMode.DoubleRow

N_CORES = 8

# xin dim-1 layout: [x0 (0:4) | wk (4:8) | wq (8:12) | wv (12:16) | x1-3 (16:28)]
XW = 28
OWK, OWQ, OWV, OX13 = 4, 8, 12, 16


def _xsl(lt):
    """dim-1 index of x l-tile `lt` (0..15) in the packed xin layout."""
    return lt if lt < CHT else OX13 - CHT + lt


def _build_attention(tc: tile.TileContext, out, xind, ctx):
    nc = tc.nc

    sb = ctx.enter_context(tc.tile_pool(name="sb", bufs=1))
    ptp = ctx.enter_context(tc.tile_pool(name="ptp", bufs=3))
    outp = ctx.enter_context(tc.tile_pool(name="outp", bufs=2))
    mmp = ctx.enter_context(tc.tile_pool(name="mmp", bufs=2, space="PSUM"))
    scp = ctx.enter_context(tc.tile_pool(name="scp", bufs=3, space="PSUM"))
    avp = ctx.enter_context(tc.tile_pool(name="avp", bufs=3, space="PSUM"))

    warm_zeros = sb.tile([P, QM], BF16)
    nc.gpsimd.memset(warm_zeros, 0.0)

    # Persistent on-chip tensors (layouts pre-built host-side):
    xall = sb.tile([P, XW, V], BF16)  # packed x + weights, v-on-partition
    qT = sb.tile([P, DT, L], FP8)     # qT[p,m,l] = q[l, m*P+p], e4m3
    kT = sb.tile([P, DT, L], FP8)
    vN = sb.tile([P, LT, D], BF16)    # vN[p,lt,d] = v[lt*P+p, d]
    acc = sb.tile([P, NQM, QM], F32)  # softmax denominator partials
    avacc = sb.tile([P, NQM * NQT, D], F32)  # AV partials (SBUF f32)
    ones_bf = sb.tile([P, 1], BF16)
    nc.gpsimd.memset(ones_bf, 1.0)

    # ---- all input DMA, emitted up front in consumption order ----
    for a, b in ((0, 5), (5, 8), (8, 12), (12, 16), (16, XW)):
        nc.sync.dma_start(out=xall[:, a:b, :], in_=xind[:, a:b, :])

    # HAM pre-warm burst while the first loads land. MUST be gapless and
    # long enough to cover the chunk-0 DMA: early TensorE gaps drop the
    # chip-wide boost clock ~20% for much of the run.
    warm_ps = mmp.tile([P, QM], F32, tag="mm")
    for _ in range(WARM):
        nc.tensor.matmul(warm_ps, lhsT=warm_zeros[:, :P], rhs=warm_zeros)

    def kq_proj(wdim, oT, m, c):
        """one [d-tile, l-window] projection chain -> fp8 (full 512)."""
        l0 = CHT * c
        x0 = _xsl(l0)
        ps = mmp.tile([P, QM], F32, tag="mm")
        for vt in range(VT):
            nc.tensor.matmul(
                ps,
                lhsT=xall[:, wdim, vt * P:(vt + 1) * P],
                rhs=xall[:, x0:x0 + CHT, vt * P:(vt + 1) * P],
                start=(vt == 0),
                stop=(vt == VT - 1),
            )
        nc.scalar.copy(out=oT[:, m, l0 * P:(l0 + CHT) * P], in_=ps)

    def v_proj(lt):
        ps = mmp.tile([P, D], F32, tag="mm")
        for vt in range(VT):
            nc.tensor.matmul(
                ps,
                lhsT=xall[:, _xsl(lt), vt * P:(vt + 1) * P],
                rhs=xall[:, OWV:OWV + DT, vt * P:(vt + 1) * P],
                start=(vt == 0),
                stop=(vt == VT - 1),
            )
        nc.scalar.copy(out=vN[:, lt, :], in_=ps)

    first_done = [False] * NQM

    def attn_pair(qm, g, split_dens=False):
        """scores+exp+denominator+AV for q-block qm against k-group g.
        split_dens: use the low-latency gp/DVE half-split denominator
        adds - only for the three pairs whose finalize follows
        immediately (the finalize cast would otherwise stall on the
        ~4.7us serial gp den chain and its FD1s block TensorE)."""
        init = not first_done[qm]
        first_done[qm] = True
        PT = ptp.tile([P, CHT, QM], BF16, tag="PT")
        H = QM // 2
        for j in range(CHT):
            kt = CHT * g + j
            ps = scp.tile([P, QM], F32, tag="sc")
            for m in (0, 2):
                nc.tensor.matmul(
                    ps,
                    lhsT=kT[:, m:m + 2, kt * P:(kt + 1) * P],
                    rhs=qT[:, m:m + 2, qm * QM:(qm + 1) * QM],
                    perf_mode=DR,
                    start=(m == 0),
                    stop=(m == 2),
                )
            nc.scalar.activation(
                out=PT[:, j, :], in_=ps,
                func=mybir.ActivationFunctionType.Exp, scale=SCALE,
            )
            # denominator accumulation on GpSimd (otherwise idle
            # mid-kernel): DVE's queue then holds only the avacc PSUM
            # drains, which otherwise lagged the 3-deep avp ring by
            # 26-517ns and stalled the next pair's first AV matmul
            # ~220ns at every chunk transition (WAR on the psum bank).
            # gp full-width adds are ~1.16us, 4 serial per pair vs the
            # pair's 5.3us of TensorE - fits with ~12% slack.
            if split_dens:
                engs = ((nc.gpsimd, slice(0, H)), (nc.vector, slice(H, QM)))
            else:
                engs = ((nc.gpsimd, slice(0, QM)),)
            for eng, sl in engs:
                if init and j == 0:
                    eng.tensor_copy(out=acc[:, qm, sl], in_=PT[:, j, sl])
                else:
                    eng.tensor_add(out=acc[:, qm, sl], in0=acc[:, qm, sl],
                                   in1=PT[:, j, sl])
        for qs in range(NQT):
            pa = avp.tile([P, D], F32, tag="av")
            for j in range(CHT):
                nc.tensor.matmul(
                    pa, lhsT=PT[:, j, qs * P:(qs + 1) * P],
                    rhs=vN[:, CHT * g + j, :],
                    start=(j == 0), stop=(j == CHT - 1),
                )
            s = qm * NQT + qs
            if init:
                nc.vector.tensor_copy(out=avacc[:, s, :], in_=pa)
            else:
                nc.vector.tensor_add(out=avacc[:, s, :], in0=avacc[:, s, :],
                                     in1=pa)

    def finalize(qm):
        """denominators -> per-partition recips -> scale+store q-block.
        The four q-tiles stage into one [P, NQT, D] tile and ship as a
        single DMA (out DRAM layout is [P, NQM, NQT, D]; the host
        transposes back, so each partition writes 8KB contiguous)."""
        acc_bf = outp.tile([P, QM], BF16, tag="acc_bf")
        nc.vector.tensor_copy(out=acc_bf, in_=acc[:, qm, :])
        zps = avp.tile([P, NQT], F32, tag="av")
        for qs in range(NQT):
            nc.tensor.matmul(zps[:, qs:qs + 1],
                             lhsT=acc_bf[:, qs * P:(qs + 1) * P],
                             rhs=ones_bf)
        zr = outp.tile([P, NQT], F32, tag="zr")
        nc.vector.reciprocal(zr, zps)
        ot4 = outp.tile([P, NQT, D], BF16, tag="ot", bufs=2)
        for qs in range(NQT):
            # scale on ScalarE (idle once exps are done; keeps the tail
            # off DVE's drain queue): out = avacc * 1/Z per-partition
            nc.scalar.activation(ot4[:, qs, :], avacc[:, qm * NQT + qs, :],
                                 mybir.ActivationFunctionType.Copy,
                                 scale=zr[:, qs:qs + 1])
        nc.sync.dma_start(out=out[:, qm, :, :], in_=ot4)

    def attn_pair_final(qm, g):
        """Last pair fused with its finalize: j-major score/AV interleave,
        denominator adds split gp/DVE as usual but the bf16 cast of the
        full denominator runs on SCALAR (idle after the exps) so the
        FD1 ones-matmuls slot in right after the last AV batch and the
        reciprocal lands ~0.4us later. Per-qs scale->store pipelines
        across DVE/ScalarE with 4 independent DMAs.

        qs=2,3 PSUM accumulators are pre-initialized with the prior
        avacc partials (DVE copy into PSUM; their AV matmuls accumulate
        with start=False) so their tail is scale-from-PSUM; qs=0,1 keep
        the SBUF-add path (the pre-init copies wouldn't land in time
        for their j0 matmuls)."""
        PT = ptp.tile([P, CHT, QM], BF16, tag="PT")
        H = QM // 2
        pas = [avp.tile([P, D], F32, tag="av", name=f"pa_fin{i}")
               for i in range(NQT - 1)]
        pas.append(mmp.tile([P, D], F32, tag="mm", name="pa_fin3"))
        zps = mmp.tile([P, NQT], F32, tag="mm")

        # pre-init qs2/qs3 accumulators (emitted first; DVE runs them
        # while TensorE is still on the j0/j1 score matmuls)
        for qs in (2, 3):
            nc.vector.tensor_copy(out=pas[qs], in_=avacc[:, qm * NQT + qs, :])

        def emit_scores(j):
            kt = CHT * g + j
            ps = scp.tile([P, QM], F32, tag="sc")
            for m in (0, 2):
                nc.tensor.matmul(
                    ps,
                    lhsT=kT[:, m:m + 2, kt * P:(kt + 1) * P],
                    rhs=qT[:, m:m + 2, qm * QM:(qm + 1) * QM],
                    perf_mode=DR,
                    start=(m == 0),
                    stop=(m == 2),
                )
            nc.scalar.activation(
                out=PT[:, j, :], in_=ps,
                func=mybir.ActivationFunctionType.Exp, scale=SCALE,
            )
            for eng, sl in ((nc.gpsimd, slice(0, H)), (nc.vector, slice(H, QM))):
                eng.tensor_add(out=acc[:, qm, sl], in0=acc[:, qm, sl],
                               in1=PT[:, j, sl])

        def emit_av(j):
            # j3 batch runs qs1 then qs0 first: their avacc adds (DVE)
            # gate the reciprocal, so their accumulators must stop as
            # early as possible.
            order = (1, 0, 2, 3) if j == CHT - 1 else range(NQT)
            for qs in order:
                nc.tensor.matmul(
                    pas[qs], lhsT=PT[:, j, qs * P:(qs + 1) * P],
                    rhs=vN[:, CHT * g + j, :],
                    start=(j == 0 and qs < 2), stop=(j == CHT - 1),
                )

        emit_scores(0)
        for j in range(CHT):
            if j + 1 < CHT:
                emit_scores(j + 1)
            emit_av(j)

        # bf16 denominator cast on ScalarE (idle after the exps), hi
        # half first: its source (DVE's den add, 426ns) lands before
        # GpSimd's lo half (728ns), so the two casts pipeline with the
        # den adds instead of waiting for both.
        acc_bf = outp.tile([P, QM], BF16, tag="acc_bf")
        nc.scalar.copy(out=acc_bf[:, H:QM], in_=acc[:, qm, H:QM])
        nc.scalar.copy(out=acc_bf[:, 0:H], in_=acc[:, qm, 0:H])
        for qs in range(NQT):
            nc.tensor.matmul(zps[:, qs:qs + 1],
                             lhsT=acc_bf[:, qs * P:(qs + 1) * P],
                             rhs=ones_bf)
        zr = outp.tile([P, NQT], F32, tag="zr")
        ot4 = outp.tile([P, NQT, D], BF16, tag="ot", bufs=2)

        def scale_sc(qs, src):
            nc.scalar.activation(ot4[:, qs, :], src,
                                 mybir.ActivationFunctionType.Copy,
                                 scale=zr[:, qs:qs + 1])

        def add_qs(qs):
            s = qm * NQT + qs
            nc.vector.tensor_add(out=avacc[:, s, :], in0=avacc[:, s, :],
                                 in1=pas[qs])

        add_qs(0)
        add_qs(1)
        nc.vector.reciprocal(zr, zps)
        nc.vector.tensor_scalar_mul(ot4[:, 0, :], avacc[:, qm * NQT + 0, :],
                                    zr[:, 0:1])
        nc.vector.tensor_scalar_mul(ot4[:, 1, :], avacc[:, qm * NQT + 1, :],
                                    zr[:, 1:2])
        scale_sc(2, pas[2])
        # two batched ships: the 4-way split left the last DIRECT2D
        # sync-queue-gated (~600ns each, serial), ending T+4.0; with
        # the now-early scales two D2Ds finish ~1.0us sooner.
        nc.sync.dma_start(out=out[:, qm, 0:2, :], in_=ot4[:, 0:2, :])
        scale_sc(3, pas[3])
        nc.sync.dma_start(out=out[:, qm, 2:NQT, :], in_=ot4[:, 2:NQT, :])

    # ---- streamed chunks ----
    for c in range(NQM):
        for wofs, oT in ((OWK, kT), (OWQ, qT)):
            for m in range(DT):
                kq_proj(wofs + m, oT, m, c)
        for lt in range(CHT * c, CHT * (c + 1)):
            v_proj(lt)
        if c < NQM - 1:
            for qm in range(c):
                attn_pair(qm, c)
            for g in range(c + 1):
                attn_pair(c, g)
        else:
            attn_pair(0, 3, split_dens=True)
            finalize(0)
            attn_pair(3, 0)
            attn_pair(1, 3, split_dens=True)
            finalize(1)
            attn_pair(3, 1)
            attn_pair(2, 3, split_dens=True)
            finalize(2)
            attn_pair(3, 2)
            attn_pair_final(3, 3)


_NC_CACHE = None


def _get_nc():
    global _NC_CACHE
    if _NC_CACHE is not None:
        return _NC_CACHE
    from contextlib import ExitStack

    nc = bacc.Bacc("TRN2", target_bir_lowering=False, debug=False,
                   num_devices=N_CORES)
    xind = nc.declare_dram_parameter("xin", [P, XW, V], BF16, isOutput=False)
    # out[p, qm, qs, d] = out_full[qm*512 + qs*128 + p, d]: each SBUF
    # partition writes contiguous DRAM per store; host transposes back
    # and upcasts. bf16 store costs ~0.03e-2 extra rel-err (simulated)
    # and halves output DMA bytes + doubles DVE scale throughput.
    out = nc.declare_dram_parameter("out", [P, NQM, NQT, D], BF16,
                                    isOutput=True)
    with tile.TileContext(nc) as tc:
        with ExitStack() as ctx:
            _build_attention(tc, out.ap(), xind.ap(), ctx)
    nc.compile()
    _NC_CACHE = nc
    return nc


def _bf16(a):
    """round-to-nearest-even f32 -> bf16 (same rounding as device cast)."""
    v = np.ascontiguousarray(a, dtype=np.float32).view(np.uint32)
    r = ((v + 0x7FFF + ((v >> 16) & 1)) >> 16).astype(np.uint16)
    return r.view(ml_dtypes.bfloat16)


def _to_vpart(a, rows_t):
    """[rows_t*P, V] -> [P, rows_t, V] with aT[p, r, vt*P+c] = a[r*P+c, vt*P+p]."""
    r4 = a.reshape(rows_t, P, VT, P)
    return np.ascontiguousarray(r4.transpose(3, 0, 2, 1).reshape(P, rows_t, V))


def _run(x, Wq, Wk, Wv, **spmd_kwargs):
    nc = _get_nc()
    x = np.asarray(x, dtype=np.float32)
    WqT = _to_vpart(_bf16(Wq), DT)
    WkT = _to_vpart(_bf16(Wk), DT)
    WvT = _to_vpart(_bf16(Wv), DT)
    in_maps = []
    for b in range(N_CORES):
        xT = _to_vpart(_bf16(x[b]), LT)
        xin = np.concatenate(
            [xT[:, 0:CHT], WkT, WqT, WvT, xT[:, CHT:LT]], axis=1)
        in_maps.append({"xin": np.ascontiguousarray(xin)})
    res = run_bass_kernel_spmd(nc, in_maps, core_ids=list(range(N_CORES)),
                               **spmd_kwargs)
    out = np.stack(
        [np.asarray(res.results[b]["out"]).transpose(1, 2, 0, 3)
         .reshape(L, D).astype(np.float32)
         for b in range(N_CORES)], axis=0)
    return out, res


def kernel(x, Wq, Wk, Wv):
    out, _ = _run(x, Wq, Wk, Wv)
    return out


# revision 68
# speedup vs baseline: 1.0214x; 1.0214x over previous
"""Fused attention kernel for Trainium2 (Bass/Tile), 8-core data-parallel.

Problem (nn_AttentionModel): B=8, L=2048, V=1024, D=512
    q = x @ Wq.T ; k = x @ Wk.T ; v = x @ Wv.T          (per batch element)
    out = softmax(q @ k.T / sqrt(D)) @ v
Sharding: data-parallel over batch - core b gets x[b] plus replicated
weights, computes its full attention on-chip, no collectives.

This revision (194us -> ~189us) adds startup + tail latency work on top
of the v1 design (host-side bf16/layout prep, flash-style streaming,
fp8 double-pumped scores - see kernel_baseline.py for that analysis).
Measured context: exec time runs from the "main" scope start to the
last instruction event; a fixed ~8.7us endgame (semaphore-clear storm,
DMA sweep, profiler flush) follows the last store regardless of kernel
structure, so the levers are (a) when useful TensorE work starts,
(b) keeping the 769-MM stream gapless, (c) when the last store lands.

1. Packed input DMA: all inputs ship as ONE DRAM tensor xin [P, 28, V]
   ordered [x_chunk0 | Wk | Wq | Wv | x_chunks1-3] in 5 sliced
   dma_starts in consumption order. Chunk-0 projections can start when
   slice [0:5] (x0 + Wk, 2.5MB... first 1.25MB critical) lands ~7.2us
   into exec. DMA descriptors run ~700ns cold vs ~270ns warm (engine
   clock ramp), so the startup transfer is ~3us/MB; finer slicing or a
   warm-up prefetch DMA measured WORSE (extra serial DIRECT2D gens).
2. Warm burst = 12 MMs sized to end exactly at chunk-0-DMA-complete
   (TensorE MMs ramp 600->216ns over ~3.5us; gaps at the start drop
   the chip-wide boost clock).
3. Fused final pair+finalize (the kernel tail): the last (qm=3, g=3)
   pair interleaves score and AV matmuls j-major (software-pipelined
   by one j), qs=2,3 PSUM accumulators are pre-initialized with the
   prior avacc partials so their tail is scale-from-PSUM, the bf16
   denominator cast runs on ScalarE right after the last exp, the four
   FD=1 ones-matmuls follow the last AV matmul, and the q-tiles scale
   across DVE (tensor_scalar muls) and ScalarE (activation scales)
   into two batched ships (four per-qs D2Ds left the last one
   sync-queue-gated, ~600ns serial each). Last store ~last-MM+4.2us.
   NOTE the tile scheduler is list-scheduling by earliest-ready time,
   NOT emission order - ops must be steered via engine choice and
   data readiness, and interleaved multi-column PSUM accumulation
   chains with manual start/stop get mangled (only canonical groups
   are safe; cost one failed revision each to learn).
4. Output stores in bf16 to a [P, NQM, NQT, D] DRAM layout (each
   partition writes contiguous bytes; host transposes + upcasts).
   Costs 0.03e-2 rel-err (simulated + verified), halves store bytes,
   and doubles DVE scale throughput. Earlier q-block finalizes batch
   all 4 tiles into one DMA; they are off the critical path.
5. Engine rebalance of the softmax denominator: regular pairs
   accumulate it entirely on GpSimd (full-width adds, ~87% gp busy),
   leaving DVE's queue holding only the avacc PSUM drains - with the
   old gp/DVE half-split, DVE lagged the 3-deep avp PSUM ring by
   26-517ns and the next pair's first AV matmul stalled ~220ns at
   every chunk transition (WAR on the psum bank). The three pairs
   whose finalize follows immediately keep the low-latency split
   (the finalize cast + FD1s would otherwise stall TensorE ~0.8us on
   the ~4.7us serial gp chain). Sharing the mm PSUM ring with pair
   accumulators instead measured catastrophically (+39us).

PSUM rings: mm 2 (projections) + sc 3 (scores) + av 3 (AV/Z) = 8 banks.
In the final pair the three avp banks hold pa[qs=0..2] simultaneously
and pa[qs=3] + zps borrow the mm ring (projections are done by then).
"""

import math
import sys

sys.path.insert(0, "/opt/trn_rl_repo")

import numpy as np
import ml_dtypes

import concourse.bacc as bacc
import concourse.bass as bass
import concourse.tile as tile
from concourse import mybir
from concourse.bass_utils import run_bass_kernel_spmd

B, L, V, D = 8, 2048, 1024, 512
P = 128
LT, VT, DT = L // P, V // P, D // P      # 16, 8, 4
QM = 512                                  # q columns per q-block
NQM = L // QM                             # 4 q-blocks == 4 chunks
NQT = QM // P                             # 4 q-tiles per block
CHT = 4                                   # l-tiles per chunk
SCALE = 1.0 / math.sqrt(D)
WARM = 12                                 # warm-burst MMs (cover chunk-0 DMA)

F32 = mybir.dt.float32
BF16 = mybir.dt.bfloat16
FP8 = mybir.dt.float8e4
DR = mybir.MatmulPerfMode.DoubleRow

N_CORES = 8

# xin dim-1 layout: [x0 (0:4) | wk (4:8) | wq (8:12) | wv (12:16) | x1-3 (16:28)]
XW = 28
OWK, OWQ, OWV, OX13 = 4, 8, 12, 16


def _xsl(lt):
    """dim-1 index of x l-tile `lt` (0..15) in the packed xin layout."""
    return lt if lt < CHT else OX13 - CHT + lt


def _build_attention(tc: tile.TileContext, out, xind, ctx):
    nc = tc.nc

    sb = ctx.enter_context(tc.tile_pool(name="sb", bufs=1))
    ptp = ctx.enter_context(tc.tile_pool(name="ptp", bufs=3))
    outp = ctx.enter_context(tc.tile_pool(name="outp", bufs=2))
    mmp = ctx.enter_context(tc.tile_pool(name="mmp", bufs=2, space="PSUM"))
    scp = ctx.enter_context(tc.tile_pool(name="scp", bufs=3, space="PSUM"))
    avp = ctx.enter_context(tc.tile_pool(name="avp", bufs=3, space="PSUM"))

    warm_zeros = sb.tile([P, QM], BF16)
    nc.gpsimd.memset(warm_zeros, 0.0)

    # Persistent on-chip tensors (layouts pre-built host-side):
    xall = sb.tile([P, XW, V], BF16)  # packed x + weights, v-on-partition
    qT = sb.tile([P, DT, L], FP8)     # qT[p,m,l] = q[l, m*P+p], e4m3
    kT = sb.tile([P, DT, L], FP8)
    vN = sb.tile([P, LT, D], BF16)    # vN[p,lt,d] = v[lt*P+p, d]
    acc = sb.tile([P, NQM, QM], F32)  # softmax denominator partials
    avacc = sb.tile([P, NQM * NQT, D], F32)  # AV partials (SBUF f32)
    ones_bf = sb.tile([P, 1], BF16)
    nc.gpsimd.memset(ones_bf, 1.0)

    # ---- all input DMA, emitted up front in consumption order ----
    for a, b in ((0, 5), (5, 8), (8, 12), (12, 16), (16, XW)):
        nc.sync.dma_start(out=xall[:, a:b, :], in_=xind[:, a:b, :])

    # HAM pre-warm burst while the first loads land. MUST be gapless and
    # long enough to cover the chunk-0 DMA: early TensorE gaps drop the
    # chip-wide boost clock ~20% for much of the run.
    warm_ps = mmp.tile([P, QM], F32, tag="mm")
    for _ in range(WARM):
        nc.tensor.matmul(warm_ps, lhsT=warm_zeros[:, :P], rhs=warm_zeros)

    def kq_proj(wdim, oT, m, c):
        """one [d-tile, l-window] projection chain -> fp8 (full 512)."""
        l0 = CHT * c
        x0 = _xsl(l0)
        ps = mmp.tile([P, QM], F32, tag="mm")
        for vt in range(VT):
            nc.tensor.matmul(
                ps,
                lhsT=xall[:, wdim, vt * P:(vt + 1) * P],
                rhs=xall[:, x0:x0 + CHT, vt * P:(vt + 1) * P],
                start=(vt == 0),
                stop=(vt == VT - 1),
            )
        nc.scalar.copy(out=oT[:, m, l0 * P:(l0 + CHT) * P], in_=ps)

    def v_proj(lt):
        ps = mmp.tile([P, D], F32, tag="mm")
        for vt in range(VT):
            nc.tensor.matmul(
                ps,
                lhsT=xall[:, _xsl(lt), vt * P:(vt + 1) * P],
                rhs=xall[:, OWV:OWV + DT, vt * P:(vt + 1) * P],
                start=(vt == 0),
                stop=(vt == VT - 1),
            )
        nc.scalar.copy(out=vN[:, lt, :], in_=ps)

    first_done = [False] * NQM

    def attn_pair(qm, g, split_dens=False):
        """scores+exp+denominator+AV for q-block qm against k-group g.
        split_dens: use the low-latency gp/DVE half-split denominator
        adds - only for the three pairs whose finalize follows
        immediately (the finalize cast would otherwise stall on the
        ~4.7us serial gp den chain and its FD1s block TensorE)."""
        init = not first_done[qm]
        first_done[qm] = True
        PT = ptp.tile([P, CHT, QM], BF16, tag="PT")
        H = QM // 2
        for j in range(CHT):
            kt = CHT * g + j
            ps = scp.tile([P, QM], F32, tag="sc")
            for m in (0, 2):
                nc.tensor.matmul(
                    ps,
                    lhsT=kT[:, m:m + 2, kt * P:(kt + 1) * P],
                    rhs=qT[:, m:m + 2, qm * QM:(qm + 1) * QM],
                    perf_mode=DR,
                    start=(m == 0),
                    stop=(m == 2),
                )
            nc.scalar.activation(
                out=PT[:, j, :], in_=ps,
                func=mybir.ActivationFunctionType.Exp, scale=SCALE,
            )
            # denominator accumulation on GpSimd (otherwise idle
            # mid-kernel): DVE's queue then holds only the avacc PSUM
            # drains, which otherwise lagged the 3-deep avp ring by
            # 26-517ns and stalled the next pair's first AV matmul
            # ~220ns at every chunk transition (WAR on the psum bank).
            # gp full-width adds are ~1.16us, 4 serial per pair vs the
            # pair's 5.3us of TensorE - fits with ~12% slack.
            if split_dens:
                engs = ((nc.gpsimd, slice(0, H)), (nc.vector, slice(H, QM)))
            else:
                engs = ((nc.gpsimd, slice(0, QM)),)
            for eng, sl in engs:
                if init and j == 0:
                    eng.tensor_copy(out=acc[:, qm, sl], in_=PT[:, j, sl])
                else:
                    eng.tensor_add(out=acc[:, qm, sl], in0=acc[:, qm, sl],
                                   in1=PT[:, j, sl])
        for qs in range(NQT):
            pa = avp.tile([P, D], F32, tag="av")
            for j in range(CHT):
                nc.tensor.matmul(
                    pa, lhsT=PT[:, j, qs * P:(qs + 1) * P],
                    rhs=vN[:, CHT * g + j, :],
                    start=(j == 0), stop=(j == CHT - 1),
                )
            s = qm * NQT + qs
            if init:
                nc.vector.tensor_copy(out=avacc[:, s, :], in_=pa)
            else:
                nc.vector.tensor_add(out=avacc[:, s, :], in0=avacc[:, s, :],
                                     in1=pa)

    def finalize(qm):
        """denominators -> per-partition recips -> scale+store q-block.
        The four q-tiles stage into one [P, NQT, D] tile and ship as a
        single DMA (out DRAM layout is [P, NQM, NQT, D]; the host
        transposes back, so each partition writes 8KB contiguous)."""
        acc_bf = outp.tile([P, QM], BF16, tag="acc_bf")
        nc.vector.tensor_copy(out=acc_bf, in_=acc[:, qm, :])
        zps = avp.tile([P, NQT], F32, tag="av")
        for qs in range(NQT):
            nc.tensor.matmul(zps[:, qs:qs + 1],
                             lhsT=acc_bf[:, qs * P:(qs + 1) * P],
                             rhs=ones_bf)
        zr = outp.tile([P, NQT], F32, tag="zr")
        nc.vector.reciprocal(zr, zps)
        ot4 = outp.tile([P, NQT, D], BF16, tag="ot", bufs=2)
        for qs in range(NQT):
            # scale on ScalarE (idle once exps are done; keeps the tail
            # off DVE's drain queue): out = avacc * 1/Z per-partition
            nc.scalar.activation(ot4[:, qs, :], avacc[:, qm * NQT + qs, :],
                                 mybir.ActivationFunctionType.Copy,
                                 scale=zr[:, qs:qs + 1])
        nc.sync.dma_start(out=out[:, qm, :, :], in_=ot4)

    def attn_pair_final(qm, g):
        """Last pair fused with its finalize: j-major score/AV interleave,
        denominator adds split gp/DVE as usual but the bf16 cast of the
        full denominator runs on SCALAR (idle after the exps) so the
        FD1 ones-matmuls slot in right after the last AV batch and the
        reciprocal lands ~0.4us later. Per-qs scale->store pipelines
        across DVE/ScalarE with 4 independent DMAs.

        qs=2,3 PSUM accumulators are pre-initialized with the prior
        avacc partials (DVE copy into PSUM; their AV matmuls accumulate
        with start=False) so their tail is scale-from-PSUM; qs=0,1 keep
        the SBUF-add path (the pre-init copies wouldn't land in time
        for their j0 matmuls)."""
        PT = ptp.tile([P, CHT, QM], BF16, tag="PT")
        H = QM // 2
        pas = [avp.tile([P, D], F32, tag="av", name=f"pa_fin{i}")
               for i in range(NQT - 1)]
        pas.append(mmp.tile([P, D], F32, tag="mm", name="pa_fin3"))
        zps = mmp.tile([P, NQT], F32, tag="mm")

        # pre-init qs2/qs3 accumulators (emitted first; DVE runs them
        # while TensorE is still on the j0/j1 score matmuls)
        for qs in (2, 3):
            nc.vector.tensor_copy(out=pas[qs], in_=avacc[:, qm * NQT + qs, :])

        def emit_scores(j):
            kt = CHT * g + j
            ps = scp.tile([P, QM], F32, tag="sc")
            for m in (0, 2):
                nc.tensor.matmul(
                    ps,
                    lhsT=kT[:, m:m + 2, kt * P:(kt + 1) * P],
                    rhs=qT[:, m:m + 2, qm * QM:(qm + 1) * QM],
                    perf_mode=DR,
                    start=(m == 0),
                    stop=(m == 2),
                )
            nc.scalar.activation(
                out=PT[:, j, :], in_=ps,
                func=mybir.ActivationFunctionType.Exp, scale=SCALE,
            )
            for eng, sl in ((nc.gpsimd, slice(0, H)), (nc.vector, slice(H, QM))):
                eng.tensor_add(out=acc[:, qm, sl], in0=acc[:, qm, sl],
                               in1=PT[:, j, sl])

        def emit_av(j):
            for qs in range(NQT):
                nc.tensor.matmul(
                    pas[qs], lhsT=PT[:, j, qs * P:(qs + 1) * P],
                    rhs=vN[:, CHT * g + j, :],
                    start=(j == 0 and qs < 2), stop=(j == CHT - 1),
                )

        emit_scores(0)
        for j in range(CHT):
            if j + 1 < CHT:
                emit_scores(j + 1)
            emit_av(j)

        # bf16 denominator cast on ScalarE (idle after the exps), hi
        # half first: its source (DVE's den add, 426ns) lands before
        # GpSimd's lo half (728ns), so the two casts pipeline with the
        # den adds instead of waiting for both.
        acc_bf = outp.tile([P, QM], BF16, tag="acc_bf")
        nc.scalar.copy(out=acc_bf[:, H:QM], in_=acc[:, qm, H:QM])
        nc.scalar.copy(out=acc_bf[:, 0:H], in_=acc[:, qm, 0:H])
        for qs in range(NQT):
            nc.tensor.matmul(zps[:, qs:qs + 1],
                             lhsT=acc_bf[:, qs * P:(qs + 1) * P],
                             rhs=ones_bf)
        zr = outp.tile([P, NQT], F32, tag="zr")
        ot4 = outp.tile([P, NQT, D], BF16, tag="ot", bufs=2)

        def scale_sc(qs, src):
            nc.scalar.activation(ot4[:, qs, :], src,
                                 mybir.ActivationFunctionType.Copy,
                                 scale=zr[:, qs:qs + 1])

        def add_qs(qs):
            s = qm * NQT + qs
            nc.vector.tensor_add(out=avacc[:, s, :], in0=avacc[:, s, :],
                                 in1=pas[qs])

        add_qs(0)
        add_qs(1)
        nc.vector.reciprocal(zr, zps)
        nc.vector.tensor_scalar_mul(ot4[:, 0, :], avacc[:, qm * NQT + 0, :],
                                    zr[:, 0:1])
        nc.vector.tensor_scalar_mul(ot4[:, 1, :], avacc[:, qm * NQT + 1, :],
                                    zr[:, 1:2])
        scale_sc(2, pas[2])
        # two batched ships: the 4-way split left the last DIRECT2D
        # sync-queue-gated (~600ns each, serial), ending T+4.0; with
        # the now-early scales two D2Ds finish ~1.0us sooner.
        nc.sync.dma_start(out=out[:, qm, 0:2, :], in_=ot4[:, 0:2, :])
        scale_sc(3, pas[3])
        nc.sync.dma_start(out=out[:, qm, 2:NQT, :], in_=ot4[:, 2:NQT, :])

    # ---- streamed chunks ----
    for c in range(NQM):
        for wofs, oT in ((OWK, kT), (OWQ, qT)):
            for m in range(DT):
                kq_proj(wofs + m, oT, m, c)
        for lt in range(CHT * c, CHT * (c + 1)):
            v_proj(lt)
        if c < NQM - 1:
            for qm in range(c):
                attn_pair(qm, c)
            for g in range(c + 1):
                attn_pair(c, g)
        else:
            attn_pair(0, 3, split_dens=True)
            finalize(0)
            attn_pair(3, 0)
            attn_pair(1, 3, split_dens=True)
            finalize(1)
            attn_pair(3, 1)
            attn_pair(2, 3, split_dens=True)
            finalize(2)
            attn_pair(3, 2)
            attn_pair_final(3, 3)


_NC_CACHE = None


def _get_nc():
    global _NC_CACHE
    if _NC_CACHE is not None:
        return _NC_CACHE
    from contextlib import ExitStack

    nc = bacc.Bacc("TRN2", target_bir_lowering=False, debug=False,
                   num_devices=N_CORES)
    xind = nc.declare_dram_parameter("xin", [P, XW, V], BF16, isOutput=False)
    # out[p, qm, qs, d] = out_full[qm*512 + qs*128 + p, d]: each SBUF
    # partition writes contiguous DRAM per store; host transposes back
    # and upcasts. bf16 store costs ~0.03e-2 extra rel-err (simulated)
    # and halves output DMA bytes + doubles DVE scale throughput.
    out = nc.declare_dram_parameter("out", [P, NQM, NQT, D], BF16,
                                    isOutput=True)
    with tile.TileContext(nc) as tc:
        with ExitStack() as ctx:
            _build_attention(tc, out.ap(), xind.ap(), ctx)
    nc.compile()
    _NC_CACHE = nc
    return nc


def _bf16(a):
    """round-to-nearest-even f32 -> bf16 (same rounding as device cast)."""
    v = np.ascontiguousarray(a, dtype=np.float32).view(np.uint32)
    r = ((v + 0x7FFF + ((v >> 16) & 1)) >> 16).astype(np.uint16)
    return r.view(ml_dtypes.bfloat16)


def _to_vpart(a, rows_t):
    """[rows_t*P, V] -> [P, rows_t, V] with aT[p, r, vt*P+c] = a[r*P+c, vt*P+p]."""
    r4 = a.reshape(rows_t, P, VT, P)
    return np.ascontiguousarray(r4.transpose(3, 0, 2, 1).reshape(P, rows_t, V))


def _run(x, Wq, Wk, Wv, **spmd_kwargs):
    nc = _get_nc()
    x = np.asarray(x, dtype=np.float32)
    WqT = _to_vpart(_bf16(Wq), DT)
    WkT = _to_vpart(_bf16(Wk), DT)
    WvT = _to_vpart(_bf16(Wv), DT)
    in_maps = []
    for b in range(N_CORES):
        xT = _to_vpart(_bf16(x[b]), LT)
        xin = np.concatenate(
            [xT[:, 0:CHT], WkT, WqT, WvT, xT[:, CHT:LT]], axis=1)
        in_maps.append({"xin": np.ascontiguousarray(xin)})
    res = run_bass_kernel_spmd(nc, in_maps, core_ids=list(range(N_CORES)),
                               **spmd_kwargs)
    out = np.stack(
        [np.asarray(res.results[b]["out"]).transpose(1, 2, 0, 3)
         .reshape(L, D).astype(np.float32)
         for b in range(N_CORES)], axis=0)
    return out, res


def kernel(x, Wq, Wk, Wv):
    out, _ = _run(x, Wq, Wk, Wv)
    return out
